# revision 1
# baseline (speedup 1.0000x reference)
"""CreditRiskGNN (2-layer GCN) Trainium2 kernel, 8 NeuronCores.

Sharding (per spec hint): nodes sharded across the 8 cores; edges partitioned
by destination node so scatter-adds are core-local; the per-shard node
features are all-gathered between layers.

Math: GCNConv(x, W, b)[d] = dinv[d] * (sum_{e: dst=d} h'[src_e] + h'[d]) + b
where h' = dinv (.) (x @ W) and dinv = rsqrt(indegree + 1) (self-loops).
Layer 2 uses (A @ R') @ W2 == A @ (R' @ W2) associativity so both layers share
one aggregation structure.

Device pipeline per core (one NEFF, SPMD on all 8 cores; per-core data only):
  A) h'_shard = dinv (.) (x_shard @ W1)        (PE matmul + DVE scale)
  B) AllGather h' -> full table [N, 64] in DRAM
  C) layer-1 aggregation per 128-dst tile: dma_gather of h'[src] rows
     (SWDGE ucode, 4 queues round-robin), one-hot dst-selection built on DVE
     (is_equal vs iota), PE matmuls accumulate into PSUM; fused epilogue
     R' = dinv (.) relu(dinv (.) (agg + self) + b1)
  D) AllGather R'
  E) layer-2 aggregation over the same edges; y = sigmoid(dinv*(agg2@W2)+b2)

Host does graph preprocessing only (CSR sharding, degree counts, gather-index
layout) and the final shard concat.
"""

import contextlib
import ctypes
import math
import os
import sys
import types

import ml_dtypes
import numpy as np

N_CORES = 8
P = 128
D_HID = 64
WIN = 32768                # int16 index window for dma_gather
MAX_IDX_PER_GATHER = 1024  # HW descriptor-ring limit (2048 hangs the queue)

LAST_RESULTS = None  # BassKernelResults of the last run (for test harnesses)


# ---------------------------------------------------------------------------
# axon NTFF profile hook shim (only needed when BASS_TRACE=1 under axon)
def _install_axon_profile_shim():
    if "antenv.axon_hooks" in sys.modules:
        return
    try:
        so_path = "/opt/axon/libaxon_pjrt.so"
        if not os.path.exists(so_path):
            return
        lib = ctypes.CDLL(so_path)
        if not hasattr(lib, "axon_start_nrt_profile"):
            return
        lib.axon_start_nrt_profile.argtypes = [
            ctypes.POINTER(ctypes.c_int64),
            ctypes.c_size_t,
        ]
        lib.axon_start_nrt_profile.restype = ctypes.c_int64
        lib.axon_stop_nrt_profile.argtypes = [ctypes.c_char_p]
        lib.axon_stop_nrt_profile.restype = ctypes.c_int64

        @contextlib.contextmanager
        def _hook(output_dir, device_ids):
            import jax

            jax.devices()
            if device_ids:
                ids = (ctypes.c_int64 * len(device_ids))(*device_ids)
                rc = lib.axon_start_nrt_profile(ids, len(device_ids))
            else:
                rc = lib.axon_start_nrt_profile(None, 0)
            if rc != 0:
                raise RuntimeError(f"axon_start_nrt_profile rc={rc}")
            try:
                yield
            finally:
                n = lib.axon_stop_nrt_profile(str(output_dir).encode())
                if n < 0:
                    raise RuntimeError(f"axon_stop_nrt_profile rc={n}")

        mod = types.ModuleType("antenv.axon_hooks")
        _state = {"hook": _hook}
        mod.set_axon_ntff_profile_hook = lambda h: _state.__setitem__("hook", h)
        mod.get_axon_ntff_profile_hook = lambda: _state["hook"]
        sys.modules["antenv.axon_hooks"] = mod
        import antenv

        antenv.axon_hooks = mod
    except Exception:
        pass


# ---------------------------------------------------------------------------
# Host-side graph preprocessing


def _wrap_idx_block(idxs_i16: np.ndarray) -> np.ndarray:
    """[n] int16 -> [128, n//16] in the SWDGE ucode layout: idx i at
    [i%16, i//16], replicated across the 8 groups of 16 partitions."""
    n = idxs_i16.shape[0]
    block = np.zeros((16, n // 16), dtype=np.int16)
    i = np.arange(n)
    block[i % 16, i // 16] = idxs_i16
    return np.tile(block, (8, 1))


def _build_plan(src, dst, n_nodes, n_cores):
    """Partition edges by destination shard; group per (dst-tile, src-window);
    pad each group to the max count across cores (rounded to 128) so the
    program shape is identical on every core."""
    sh = n_nodes // n_cores
    n_tiles = math.ceil(sh / P)
    n_win = math.ceil(n_nodes / WIN)

    core_of = dst // sh
    counts = np.zeros((n_cores, n_tiles, n_win), dtype=np.int64)
    per_core_sorted = []
    for c in range(n_cores):
        m = core_of == c
        s_c = src[m].astype(np.int64)
        d_c = (dst[m] - c * sh).astype(np.int64)
        tile_id = d_c // P
        win = s_c // WIN
        order = np.lexsort((s_c, win, tile_id))
        s_c, d_c = s_c[order], d_c[order]
        key = (d_c // P) * n_win + (s_c // WIN)
        allkeys = np.arange(n_tiles * n_win)
        starts = np.searchsorted(key, allkeys, side="left").reshape(n_tiles, n_win)
        ends = np.searchsorted(key, allkeys, side="right").reshape(n_tiles, n_win)
        counts[c] = ends - starts
        per_core_sorted.append((s_c, d_c, starts))

    padded = counts.max(axis=0)
    padded = np.where(padded > 0, ((padded + 15) // 16) * 16, 0).astype(np.int64)

    groups = []  # (t, w, ni, idx_off16, chunk_off)
    tile_nch = np.zeros(n_tiles, dtype=np.int64)
    tile_chunk_off = np.zeros(n_tiles, dtype=np.int64)
    off16 = 0
    chunk_off = 0
    for t in range(n_tiles):
        tile_chunk_off[t] = chunk_off
        for w in range(n_win):
            ni = int(padded[t, w])
            if ni == 0:
                continue
            groups.append((t, w, ni, off16, chunk_off))
            off16 += ni // 16
            chunk_off += (ni + P - 1) // P
        tile_nch[t] = chunk_off - tile_chunk_off[t]

    chunk_k = np.zeros(int(chunk_off), dtype=np.int64)
    for (t, w, ni, o16, ch_off) in groups:
        ncg = (ni + P - 1) // P
        for k in range(ncg):
            chunk_k[ch_off + k] = min(P, ni - k * P)

    meta = dict(
        n_nodes=n_nodes,
        sh=sh,
        n_tiles=n_tiles,
        n_win=n_win,
        groups=groups,
        tile_nch=tile_nch,
        tile_chunk_off=tile_chunk_off,
        chunk_k=chunk_k,
        total_idx=off16 * 16,
        total_chunks=int(chunk_off),
    )

    per_core = []
    for c in range(n_cores):
        s_c, d_c, starts = per_core_sorted[c]
        idx_arr = np.zeros((P, meta["total_idx"] // 16), dtype=np.int16)
        dst_arr = np.full((P, meta["total_chunks"]), -1.0, dtype=np.float32)
        for (t, w, ni, o16, ch_off) in groups:
            n_real = int(counts[c, t, w])
            st = int(starts[t, w])
            rel = np.zeros(ni, dtype=np.int16)
            if n_real > 0:
                rel[:n_real] = (s_c[st : st + n_real] - w * WIN).astype(np.int16)
            idx_arr[:, o16 : o16 + ni // 16] = _wrap_idx_block(rel)
            if n_real > 0:
                i = np.arange(n_real)
                dst_arr[i % P, ch_off + i // P] = (d_c[st : st + n_real] % P).astype(
                    np.float32
                )
        per_core.append((idx_arr, dst_arr))
    return meta, per_core


# ---------------------------------------------------------------------------
# Device program


def _build_program(meta):
    import concourse.bacc as bacc
    import concourse.mybir as mybir
    import concourse.tile as tile

    n_nodes = meta["n_nodes"]
    sh = meta["sh"]
    n_tiles = meta["n_tiles"]
    groups = meta["groups"]
    tile_nch = meta["tile_nch"]
    tile_chunk_off = meta["tile_chunk_off"]
    chunk_k = meta["chunk_k"]
    total_idx = meta["total_idx"]
    total_chunks = meta["total_chunks"]

    f32 = mybir.dt.float32
    nc = bacc.Bacc("TRN2", target_bir_lowering=False, debug=False, num_swdge_queues=4)

    xT = nc.dram_tensor("xT", [P, sh], f32, kind="ExternalInput")
    w1 = nc.dram_tensor("w1", [P, D_HID], f32, kind="ExternalInput")
    b1bc = nc.dram_tensor("b1bc", [P, D_HID], f32, kind="ExternalInput")
    w2bc = nc.dram_tensor("w2bc", [P, D_HID], f32, kind="ExternalInput")
    dinv_sh = nc.dram_tensor("dinv_sh", [P, n_tiles], f32, kind="ExternalInput")
    iota = nc.dram_tensor("iota", [P, P], f32, kind="ExternalInput")
    iota4 = nc.dram_tensor("iota4", [P, 4, P], mybir.dt.bfloat16, kind="ExternalInput")
    idx16 = nc.dram_tensor(
        "idx16", [P, total_idx // 16], mybir.dt.int16, kind="ExternalInput"
    )
    dstloc = nc.dram_tensor("dstloc", [P, total_chunks], mybir.dt.bfloat16, kind="ExternalInput")
    b2col = nc.dram_tensor("b2col", [P, 1], f32, kind="ExternalInput")
    y_out = nc.dram_tensor("y", [sh, 1], f32, kind="ExternalOutput")

    h_sh = nc.dram_tensor("h_sh", [sh, D_HID], f32, kind="Internal")
    h_full = nc.dram_tensor(
        "h_full", [n_nodes, D_HID], f32, kind="Internal", addr_space="Shared"
    )
    r_sh = nc.dram_tensor("r_sh", [sh, D_HID], f32, kind="Internal")
    r_full = nc.dram_tensor(
        "r_full", [n_nodes, D_HID], f32, kind="Internal", addr_space="Shared"
    )

    rg = [list(range(N_CORES))]

    with tile.TileContext(nc) as tc:
        with (
            tc.tile_pool(name="const", bufs=1) as cpool,
            tc.tile_pool(name="sbuf", bufs=1) as pool,
            tc.tile_pool(name="psum", bufs=1, space="PSUM") as psum_pool,
        ):
            w1_t = cpool.tile([P, D_HID], f32)
            nc.sync.dma_start(w1_t[:], w1[:])
            b1_t = cpool.tile([P, D_HID], f32)
            nc.sync.dma_start(b1_t[:], b1bc[:])
            w2_t = cpool.tile([P, D_HID], f32)
            nc.sync.dma_start(w2_t[:], w2bc[:])
            dinv_t = cpool.tile([P, n_tiles], f32)
            nc.sync.dma_start(dinv_t[:], dinv_sh[:])
            iota_t = cpool.tile([P, P], f32)
            nc.sync.dma_start(iota_t[:], iota[:])
            iota4_t = cpool.tile([P, 4, P], mybir.dt.bfloat16)
            nc.sync.dma_start(iota4_t[:], iota4[:])
            idx_t = cpool.tile([P, total_idx // 16], mybir.dt.int16)
            nc.sync.dma_start(idx_t[:], idx16[:])
            dl_t = cpool.tile([P, total_chunks], mybir.dt.bfloat16)
            nc.sync.dma_start(dl_t[:], dstloc[:])
            b2_t = cpool.tile([P, 1], f32)
            nc.sync.dma_start(b2_t[:], b2col[:])

            # ---- phase A: h' = dinv (.) (x @ W1) -> h_sh
            # 4 node-tiles per DMA to amortize HWDGE fixed cost
            B4 = 4
            for t4 in range(0, n_tiles, B4):
                nb = min(B4, n_tiles - t4)
                c0 = t4 * P
                cn = min(sh, (t4 + B4) * P) - c0
                xt = pool.tile([P, B4 * P], f32, tag="xt", bufs=3)
                nc.sync.dma_start(xt[:, :cn], xT[:, c0 : c0 + cn])
                hs4 = pool.tile([P, B4, D_HID], f32, tag="hs", bufs=3)
                for j in range(nb):
                    t = t4 + j
                    pt = min(P, sh - t * P)
                    ph = psum_pool.tile(
                        [P, D_HID], f32, tag="ph", bufs=2, space="PSUM"
                    )
                    nc.tensor.matmul(
                        ph[:pt, :],
                        lhsT=xt[:, j * P : j * P + pt],
                        rhs=w1_t[:],
                        start=True,
                        stop=True,
                    )
                    nc.vector.tensor_scalar_mul(
                        hs4[:pt, j, :], ph[:pt, :], dinv_t[:pt, t : t + 1]
                    )
                if cn == nb * P:
                    dst_ap = h_sh[c0 : c0 + cn, :].rearrange(
                        "(j p) d -> p j d", p=P
                    )
                    nc.sync.dma_start(dst_ap, hs4[:, :nb, :])
                else:
                    for j in range(nb):
                        t = t4 + j
                        pt = min(P, sh - t * P)
                        nc.sync.dma_start(
                            h_sh[t * P : t * P + pt, :], hs4[:pt, j, :]
                        )

            # ---- phase B: AllGather h'
            nc.gpsimd.collective_compute(
                "AllGather",
                mybir.AluOpType.bypass,
                replica_groups=rg,
                ins=[h_sh[:]],
                outs=[h_full[:]],
            )

            qn_state = [0]

            def agg_layer(table, self_src, layer):
                for t in range(n_tiles):
                    pt = min(P, sh - t * P)
                    nch = int(tile_nch[t])
                    ch0 = int(tile_chunk_off[t])
                    if nch > 0:
                        gbuf = pool.tile(
                            [P, nch, D_HID], f32, tag=f"g{layer}", bufs=3
                        )
                        gbf = pool.tile(
                            [P, nch, D_HID], mybir.dt.bfloat16, tag=f"gb{layer}", bufs=3
                        )
                        col = 0
                        for (gt, w, ni, o16, ch_off) in groups:
                            if gt != t:
                                continue
                            base = w * WIN
                            span = min(WIN, n_nodes - base)
                            done = 0
                            while done < ni:
                                take = min(MAX_IDX_PER_GATHER, ni - done)
                                ncg = (take + P - 1) // P
                                nc.gpsimd.dma_gather(
                                    gbuf[:, col : col + ncg, :],
                                    table[base : base + span, :],
                                    idx_t[
                                        :,
                                        o16 + done // 16 : o16 + (done + take) // 16,
                                    ],
                                    take,
                                    take,
                                    D_HID,
                                    queue_num=qn_state[0] % 4,
                                )
                                qn_state[0] += 1
                                done += take
                                col += ncg
                    if nch > 0:
                        nc.scalar.copy(out=gbf[:], in_=gbuf[:])
                    st = pool.tile([P, D_HID], f32, tag=f"st{layer}", bufs=3)
                    if pt < P:
                        nc.vector.memset(st[:], 0.0)
                    nc.sync.dma_start(st[:pt, :], self_src[t * P : t * P + pt, :])
                    if nch > 0:
                        pa = psum_pool.tile(
                            [P, D_HID], f32, tag=f"pa{layer}", bufs=2, space="PSUM"
                        )
                        for cb in range(0, nch, 4):
                            b = min(4, nch - cb)
                            oh = pool.tile([P, 4, P], mybir.dt.bfloat16, tag=f"oh{layer}", bufs=4)
                            dls = dl_t[:, ch0 + cb : ch0 + cb + b].rearrange(
                                "p (b o) -> p b o", o=1
                            )
                            nc.vector.tensor_tensor(
                                out=oh[:, :b, :],
                                in0=dls.to_broadcast([P, b, P]),
                                in1=iota4_t[:, :b, :],
                                op=mybir.AluOpType.is_equal,
                            )
                            for k in range(b):
                                ch = cb + k
                                kk = int(chunk_k[ch0 + ch])
                                nc.tensor.matmul(
                                    pa[:],
                                    lhsT=oh[:kk, k, :],
                                    rhs=gbf[:kk, ch, :],
                                    start=(ch == 0),
                                    stop=(ch == nch - 1),
                                )
                    dv = dinv_t[:pt, t : t + 1]
                    if layer == 1:
                        t1 = pool.tile([P, D_HID], f32, tag="t1", bufs=3)
                        if nch > 0:
                            nc.vector.tensor_add(t1[:pt, :], pa[:pt, :], st[:pt, :])
                        else:
                            nc.vector.tensor_copy(out=t1[:pt, :], in_=st[:pt, :])
                        t2 = pool.tile([P, D_HID], f32, tag="t2", bufs=3)
                        nc.vector.tensor_scalar_mul(t2[:pt, :], t1[:pt, :], dv)
                        t3 = pool.tile([P, D_HID], f32, tag="t3", bufs=3)
                        nc.vector.tensor_add(t3[:pt, :], t2[:pt, :], b1_t[:pt, :])
                        rr = pool.tile([P, D_HID], f32, tag="rr", bufs=3)
                        nc.scalar.activation(
                            rr[:pt, :], t3[:pt, :], mybir.ActivationFunctionType.Relu
                        )
                        rp = pool.tile([P, D_HID], f32, tag="rp", bufs=3)
                        nc.vector.tensor_scalar_mul(rp[:pt, :], rr[:pt, :], dv)
                        nc.sync.dma_start(r_sh[t * P : t * P + pt, :], rp[:pt, :])
                    else:
                        u1 = pool.tile([P, D_HID], f32, tag="u1", bufs=3)
                        if nch > 0:
                            nc.vector.tensor_add(u1[:pt, :], pa[:pt, :], st[:pt, :])
                        else:
                            nc.vector.tensor_copy(out=u1[:pt, :], in_=st[:pt, :])
                        u2 = pool.tile([P, D_HID], f32, tag="u2", bufs=3)
                        nc.vector.tensor_mul(u2[:pt, :], u1[:pt, :], w2_t[:pt, :])
                        yv = pool.tile([P, 1], f32, tag="yv", bufs=3)
                        nc.vector.tensor_reduce(
                            yv[:pt, :],
                            u2[:pt, :],
                            axis=mybir.AxisListType.X,
                            op=mybir.AluOpType.add,
                        )
                        ov = pool.tile([P, 1], f32, tag="ov", bufs=3)
                        nc.scalar.activation(
                            ov[:pt, :],
                            yv[:pt, :],
                            mybir.ActivationFunctionType.Sigmoid,
                            bias=b2_t[:pt, :],
                            scale=dv,
                        )
                        nc.sync.dma_start(y_out[t * P : t * P + pt, :], ov[:pt, :])

            # ---- phase C: layer 1 (table = h_full, self rows = local h_sh)
            agg_layer(h_full, h_sh, layer=1)

            # ---- phase D: AllGather R'
            nc.gpsimd.collective_compute(
                "AllGather",
                mybir.AluOpType.bypass,
                replica_groups=rg,
                ins=[r_sh[:]],
                outs=[r_full[:]],
            )

            # ---- phase E: layer 2
            agg_layer(r_full, r_sh, layer=2)

    nc.compile()
    return nc


# ---------------------------------------------------------------------------


def kernel(**inputs) -> np.ndarray:
    global LAST_RESULTS
    x = np.asarray(inputs["x"], dtype=np.float32)
    edge_index = np.asarray(inputs["edge_index"])
    w1_in = np.asarray(inputs["W1"], dtype=np.float32)
    b1_in = np.asarray(inputs["b1"], dtype=np.float32)
    w2_in = np.asarray(inputs["W2"], dtype=np.float32)
    b2_in = np.asarray(inputs["b2"], dtype=np.float32)

    n_nodes = x.shape[0]
    src = edge_index[0].astype(np.int64)
    dst = edge_index[1].astype(np.int64)

    deg = np.bincount(dst, minlength=n_nodes).astype(np.float64) + 1.0
    dinv = (1.0 / np.sqrt(deg)).astype(np.float32)

    meta, per_core = _build_plan(src, dst, n_nodes, N_CORES)
    sh = meta["sh"]
    n_tiles = meta["n_tiles"]

    nc = _build_program(meta)

    iota_arr = np.broadcast_to(np.arange(P, dtype=np.float32), (P, P)).copy()
    iota4_arr = (
        np.broadcast_to(np.arange(P, dtype=np.float32), (P, 4, P))
        .astype(ml_dtypes.bfloat16)
        .copy()
    )
    b1bc = np.broadcast_to(b1_in.reshape(1, D_HID), (P, D_HID)).copy()
    w2bc = np.broadcast_to(w2_in.reshape(1, D_HID), (P, D_HID)).copy()

    in_maps = []
    for c in range(N_CORES):
        idx_arr, dst_arr = per_core[c]
        xs = x[c * sh : (c + 1) * sh]  # [sh, 128]
        xT = np.ascontiguousarray(xs.T)  # [128, sh]
        dv = np.zeros((P, n_tiles), dtype=np.float32)
        dsl = dinv[c * sh : (c + 1) * sh]
        for t in range(n_tiles):
            pt = min(P, sh - t * P)
            dv[:pt, t] = dsl[t * P : t * P + pt]
        in_maps.append(
            {
                "xT": xT,
                "w1": w1_in,
                "b1bc": b1bc,
                "w2bc": w2bc,
                "dinv_sh": dv,
                "iota": iota_arr,
                "iota4": iota4_arr,
                "idx16": idx_arr,
                "dstloc": dst_arr.astype(ml_dtypes.bfloat16),
                "b2col": np.full((P, 1), float(b2_in.reshape(-1)[0]), dtype=np.float32),
            }
        )

    from concourse import bass_utils

    if os.environ.get("BASS_TRACE"):
        _install_axon_profile_shim()

    res = bass_utils.run_bass_kernel_spmd(
        nc,
        in_maps,
        core_ids=list(range(N_CORES)),
        trace=bool(os.environ.get("BASS_TRACE")),
        trace_cores=[0] if os.environ.get("BASS_TRACE") else None,
    )
    LAST_RESULTS = res
    out = np.concatenate([res.results[c]["y"] for c in range(N_CORES)], axis=0)
    return out.astype(np.float32)



# revision 5
# speedup vs baseline: 1.3343x; 1.3343x over previous
"""CreditRiskGNN (2-layer GCN) Trainium2 kernel, 8 NeuronCores.

Structure (v2): the GCN linear transform commutes with the (linear)
neighbor aggregation, so layer 1 gathers pre-scaled input rows
x~ = dinv (.) x directly (bf16 [N,128] = one 256B SWDGE row per edge) and
applies W1 per dst-tile AFTER aggregation:

  L1:  A1[d] = sum_{e: dst=d} x~[src_e]  (+ x~[d] self)      (PE one-hot)
       r'[d] = dinv[d] * relu(dinv[d] * (A1[d] @ W1) + b1)   (PE + epilogue)
  AG:  AllGather r' (padded bf16 [N,128] rows)
  L2:  A2[d] = sum r'[src_e] + r'[d];  y = sigmoid(dinv*(A2.W2)+b2)

This removes the phase-A matmul, the first AllGather, and all f32->bf16
copies of gathered data (tables are bf16; gathered chunks feed the PE
directly). The binding resource is SWDGE descriptor generation
(~2.2ns/row aggregate with 4 queues); gathers are packed into 1024-idx
calls (measured: per-call cost is ring-drain bound, so bigger calls and
deep buffering matter, row bytes don't: 512B rows drain at 256B rate).

Edges are partitioned by dst core; per core they are grouped by
(super-round of 12 dst tiles, src window of 32768, dst tile) so PSUM can
hold all 12 accumulators [128 feat, 128 dst] of a round. Chunk columns
shared by two tiles are consumed twice with -1-masked dstloc columns.
"""

import contextlib
import ctypes
import math
import os
import sys
import types

import ml_dtypes
import numpy as np

N_CORES = 8
P = 128
D_IN = 128
D_HID = 64
WIN = 32768
MAX_CALL = 1024         # HW limit: >1024 idx per dma_gather hangs the queue
ROUND_TILES = 6         # psum accumulators use a full 2KB bank each (8 banks)

LAST_RESULTS = None


# ---------------------------------------------------------------------------
# axon NTFF profile hook shim (only needed when BASS_TRACE=1 under axon)
def _install_axon_profile_shim():
    if "antenv.axon_hooks" in sys.modules:
        return
    try:
        so_path = "/opt/axon/libaxon_pjrt.so"
        if not os.path.exists(so_path):
            return
        lib = ctypes.CDLL(so_path)
        if not hasattr(lib, "axon_start_nrt_profile"):
            return
        lib.axon_start_nrt_profile.argtypes = [
            ctypes.POINTER(ctypes.c_int64),
            ctypes.c_size_t,
        ]
        lib.axon_start_nrt_profile.restype = ctypes.c_int64
        lib.axon_stop_nrt_profile.argtypes = [ctypes.c_char_p]
        lib.axon_stop_nrt_profile.restype = ctypes.c_int64

        @contextlib.contextmanager
        def _hook(output_dir, device_ids):
            import jax

            jax.devices()
            if device_ids:
                ids = (ctypes.c_int64 * len(device_ids))(*device_ids)
                rc = lib.axon_start_nrt_profile(ids, len(device_ids))
            else:
                rc = lib.axon_start_nrt_profile(None, 0)
            if rc != 0:
                raise RuntimeError(f"axon_start_nrt_profile rc={rc}")
            try:
                yield
            finally:
                n = lib.axon_stop_nrt_profile(str(output_dir).encode())
                if n < 0:
                    raise RuntimeError(f"axon_stop_nrt_profile rc={n}")

        mod = types.ModuleType("antenv.axon_hooks")
        _state = {"hook": _hook}
        mod.set_axon_ntff_profile_hook = lambda h: _state.__setitem__("hook", h)
        mod.get_axon_ntff_profile_hook = lambda: _state["hook"]
        sys.modules["antenv.axon_hooks"] = mod
        import antenv

        antenv.axon_hooks = mod
    except Exception:
        pass


# ---------------------------------------------------------------------------
# Host-side graph preprocessing


def _wrap_idx_block(idxs_i16: np.ndarray) -> np.ndarray:
    """[n] int16 -> [128, n//16] SWDGE ucode layout: idx i at [i%16, i//16],
    replicated across the 8 groups of 16 partitions."""
    n = idxs_i16.shape[0]
    block = np.zeros((16, n // 16), dtype=np.int16)
    i = np.arange(n)
    block[i % 16, i // 16] = idxs_i16
    return np.tile(block, (8, 1))


def _build_plan(src, dst, n_nodes, n_cores):
    """Group edges per (round, window, tile), pad each run to the max count
    across cores (rounded to 16) so the SPMD program shape is identical.

    Returns meta (program-shape constants) and per-core (idx16, dstloc).
    meta['calls']: list of (w, base_off16, take, ncg) in issue order, where
      base positions index the (r,w) group stream.
    meta['entries']: per call index, list of consumption entries
      (ch_in_call, kk, tile, col, start, stop).
    """
    sh = n_nodes // n_cores
    n_tiles = math.ceil(sh / P)
    n_win = math.ceil(n_nodes / WIN)
    n_rounds = math.ceil(n_tiles / ROUND_TILES)

    core_of = dst // sh
    counts = np.zeros((n_cores, n_rounds, n_win, n_tiles), dtype=np.int64)
    per_core_sorted = []
    for c in range(n_cores):
        m = core_of == c
        s_c = src[m].astype(np.int64)
        d_c = (dst[m] - c * sh).astype(np.int64)
        t_c = d_c // P
        w_c = s_c // WIN
        r_c = t_c // ROUND_TILES
        order = np.lexsort((s_c, t_c, w_c, r_c))
        s_c, d_c, t_c, w_c, r_c = (
            s_c[order], d_c[order], t_c[order], w_c[order], r_c[order])
        key = (r_c * n_win + w_c) * n_tiles + t_c
        allkeys = np.arange(n_rounds * n_win * n_tiles)
        st = np.searchsorted(key, allkeys, side="left")
        en = np.searchsorted(key, allkeys, side="right")
        counts[c] = (en - st).reshape(n_rounds, n_win, n_tiles)
        per_core_sorted.append((s_c, d_c, st.reshape(n_rounds, n_win, n_tiles)))

    padded = counts.max(axis=0)
    padded = ((padded + 15) // 16) * 16  # %16 idx-wrap granularity (0 stays 0)

    # ---- program-shape metadata (same for all cores)
    calls = []          # (w_global, off16_in_idxarr, take, ncg, ch0_global)
    entries_per_call = []
    col_ctr = 0
    off16 = 0
    ch_global = 0
    tile_first = {}     # tile -> (call_i, entry_i) first/last for start/stop
    tile_last = {}
    for r in range(n_rounds):
        for w in range(n_win):
            G = int(padded[r, w].sum())
            if G == 0:
                continue
            # tile run boundaries within the group stream
            runs = np.cumsum(np.concatenate([[0], padded[r, w]]))
            n_ch = (G + P - 1) // P
            # chunk -> consuming tiles
            cons_by_ch = [[] for _ in range(n_ch)]
            for t in range(n_tiles):
                a, b = int(runs[t]), int(runs[t + 1])
                if a == b:
                    continue
                for ch in range(a // P, (b - 1) // P + 1):
                    cons_by_ch[ch].append(t)
            # calls split at 1024 boundaries
            done = 0
            while done < G:
                take = min(MAX_CALL, G - done)
                ncg = (take + P - 1) // P
                ent = []
                for chl in range(ncg):
                    ch = done // P + chl
                    kk = min(P, G - ch * P)
                    for t in cons_by_ch[ch]:
                        ent.append([chl, kk, t, col_ctr, False, False])
                        tk = (r, t)
                        if tk not in tile_first:
                            tile_first[tk] = (len(calls), len(ent) - 1)
                        tile_last[tk] = (len(calls), len(ent) - 1)
                        col_ctr += 1
                calls.append((w, off16 + done // 16, take, ncg, r))
                entries_per_call.append(ent)
                done += take
            off16 += G // 16
            ch_global += n_ch
    for (r, t), (ci, ei) in tile_first.items():
        entries_per_call[ci][ei][4] = True
    for (r, t), (ci, ei) in tile_last.items():
        entries_per_call[ci][ei][5] = True

    total_idx = off16 * 16
    n_cols = col_ctr

    meta = dict(
        n_nodes=n_nodes, sh=sh, n_tiles=n_tiles, n_win=n_win,
        n_rounds=n_rounds, calls=calls, entries=entries_per_call,
        total_idx=total_idx, n_cols=n_cols,
    )

    # ---- per-core idx / dstloc arrays
    per_core = []
    for c in range(n_cores):
        s_c, d_c, starts = per_core_sorted[c]
        idx_arr = np.zeros((P, total_idx // 16), dtype=np.int16)
        dst_arr = np.full((P, n_cols), -1.0, dtype=np.float32)
        off16c = 0
        colc = 0
        for r in range(n_rounds):
            for w in range(n_win):
                G = int(padded[r, w].sum())
                if G == 0:
                    continue
                gs = np.zeros(G, dtype=np.int16)
                gd = np.full(G, -1.0, dtype=np.float32)
                pos = 0
                for t in range(n_tiles):
                    ni = int(padded[r, w, t])
                    if ni == 0:
                        continue
                    nr = int(counts[c, r, w, t])
                    a = int(starts[r, w, t])
                    gs[pos:pos + nr] = (s_c[a:a + nr] - w * WIN).astype(np.int16)
                    gd[pos:pos + nr] = (d_c[a:a + nr] - t * P).astype(np.float32)
                    pos += ni
                idx_arr[:, off16c:off16c + G // 16] = _wrap_idx_block(gs)
                # dstloc columns in consumption (col) order
                runs = np.cumsum(np.concatenate([[0], padded[r, w]]))
                n_ch = (G + P - 1) // P
                for ch in range(n_ch):
                    kk = min(P, G - ch * P)
                    for t in range(n_tiles):
                        a, b = int(runs[t]), int(runs[t + 1])
                        if a == b or ch < a // P or ch > (b - 1) // P:
                            continue
                        colv = np.full(P, -1.0, dtype=np.float32)
                        lo, hi = ch * P, ch * P + kk
                        sel = np.arange(max(lo, a), min(hi, b))
                        if len(sel):
                            colv[sel - lo] = gd[sel]
                        dst_arr[:, colc] = colv
                        colc += 1
                off16c += G // 16
        per_core.append((idx_arr, dst_arr.astype(ml_dtypes.bfloat16)))
    return meta, per_core


# ---------------------------------------------------------------------------
# Device program


def _build_program(meta):
    import concourse.bacc as bacc
    import concourse.mybir as mybir
    import concourse.tile as tile

    n_nodes = meta["n_nodes"]
    sh = meta["sh"]
    n_tiles = meta["n_tiles"]
    calls = meta["calls"]
    entries = meta["entries"]
    total_idx = meta["total_idx"]
    n_cols = meta["n_cols"]

    f32 = mybir.dt.float32
    bf16 = mybir.dt.bfloat16
    i16 = mybir.dt.int16
    AF = mybir.ActivationFunctionType

    nc = bacc.Bacc("TRN2", target_bir_lowering=False, debug=False,
                   num_swdge_queues=4)

    xsb = nc.dram_tensor("xsb", [n_nodes, D_IN], bf16, kind="ExternalInput")
    xsTb = nc.dram_tensor("xsTb", [P, sh], bf16, kind="ExternalInput")
    w1b = nc.dram_tensor("w1b", [P, D_HID], bf16, kind="ExternalInput")
    b1bc = nc.dram_tensor("b1bc", [P, D_HID], f32, kind="ExternalInput")
    w2bc = nc.dram_tensor("w2bc", [P, D_HID], f32, kind="ExternalInput")
    dinv_sh = nc.dram_tensor("dinv_sh", [P, n_tiles], f32, kind="ExternalInput")
    iota4 = nc.dram_tensor("iota4", [P, 4, P], bf16, kind="ExternalInput")
    idx16 = nc.dram_tensor("idx16", [P, total_idx // 16], i16, kind="ExternalInput")
    dstloc = nc.dram_tensor("dstloc", [P, n_cols], bf16, kind="ExternalInput")
    b2col = nc.dram_tensor("b2col", [P, 1], f32, kind="ExternalInput")
    y_out = nc.dram_tensor("y", [sh, 1], f32, kind="ExternalOutput")

    r_shp = nc.dram_tensor("r_shp", [sh, P], bf16, kind="Internal")
    r_fullp = nc.dram_tensor(
        "r_fullp", [n_nodes, P], bf16, kind="Internal", addr_space="Shared")

    rg = [list(range(N_CORES))]
    qn = [0]

    with tile.TileContext(nc) as tc:
        with (
            tc.tile_pool(name="const", bufs=1) as cpool,
            tc.tile_pool(name="sbuf", bufs=1) as pool,
            tc.tile_pool(name="psum", bufs=1, space="PSUM") as psum_pool,
        ):
            idx_t = cpool.tile([P, total_idx // 16], i16)
            nc.sync.dma_start(idx_t[:], idx16[:])
            dl_t = cpool.tile([P, n_cols], bf16)
            nc.sync.dma_start(dl_t[:], dstloc[:])
            iota4_t = cpool.tile([P, 4, P], bf16)
            nc.sync.dma_start(iota4_t[:], iota4[:])
            w1_t = cpool.tile([P, D_HID], bf16)
            nc.sync.dma_start(w1_t[:], w1b[:])
            b1_t = cpool.tile([P, D_HID], f32)
            nc.sync.dma_start(b1_t[:], b1bc[:])
            w2_t = cpool.tile([P, D_HID], f32)
            nc.sync.dma_start(w2_t[:], w2bc[:])
            dinv_t = cpool.tile([P, n_tiles], f32)
            nc.sync.dma_start(dinv_t[:], dinv_sh[:])
            xsT_t = cpool.tile([P, sh], bf16)
            nc.sync.dma_start(xsT_t[:], xsTb[:])
            b2_t = cpool.tile([P, 1], f32)
            nc.sync.dma_start(b2_t[:], b2col[:])

            def gather_layer(layer, table, table_rows):
                """Issue calls + consumption for one layer; psum accum per
                tile; returns dict tile -> psum tile for epilogue hookup."""
                px = {}
                cur_round = [-1]

                def flush_round(r_new):
                    # epilogue for tiles of the finished round
                    if cur_round[0] >= 0:
                        for t in sorted(px):
                            epilogue(layer, t, px.pop(t))
                    cur_round[0] = r_new

                for ci, (w, off16, take, ncg, r) in enumerate(calls):
                    if r != cur_round[0]:
                        flush_round(r)
                    base = w * WIN
                    span = min(WIN, table_rows - base)
                    g = pool.tile([P, 8, D_IN], bf16, tag=f"gx{layer}", bufs=10)
                    nc.gpsimd.dma_gather(
                        g[:, :ncg, :], table[base:base + span, :],
                        idx_t[:, off16:off16 + take // 16], take, take, D_IN,
                        queue_num=qn[0] % 4)
                    qn[0] += 1
                    ent = entries[ci]
                    for eb in range(0, len(ent), 4):
                        blk = ent[eb:eb + 4]
                        b = len(blk)
                        c0 = blk[0][3]
                        # columns in a block are consecutive by construction
                        oh = pool.tile([P, 4, P], bf16, tag=f"oh{layer}", bufs=8)
                        dls = dl_t[:, c0:c0 + b].rearrange("p (b o) -> p b o", o=1)
                        nc.vector.tensor_tensor(
                            out=oh[:, :b, :],
                            in0=dls.to_broadcast([P, b, P]),
                            in1=iota4_t[:, :b, :],
                            op=mybir.AluOpType.is_equal)
                        for j, (chl, kk, t, col, st_f, sp_f) in enumerate(blk):
                            if t not in px:
                                pxt_new = psum_pool.tile(
                                    [P, D_IN], f32, tag="px",
                                    bufs=ROUND_TILES, space="PSUM",
                                    name=f"px{layer}_{t}")
                                px[t] = pxt_new
                            if layer == 1:
                                nc.tensor.matmul(
                                    px[t][:],
                                    lhsT=g[:kk, chl, :],
                                    rhs=oh[:kk, j, :],
                                    start=st_f, stop=sp_f)
                            else:
                                nc.tensor.matmul(
                                    px[t][:, 0:D_HID],
                                    lhsT=oh[:kk, j, :],
                                    rhs=g[:kk, chl, 0:D_HID],
                                    start=st_f, stop=sp_f)
                flush_round(-1)

            def epilogue(layer, t, pxt):
                pt = min(P, sh - t * P)
                dv = dinv_t[:pt, t:t + 1]
                if layer == 1:
                    # PSX = bf16 copy of psum_x [feat, dst]
                    psx = pool.tile([P, P], bf16, tag="psx", bufs=4)
                    nc.scalar.activation(psx[:], pxt[:], AF.Copy)
                    p2 = psum_pool.tile([P, D_HID], f32, tag="p2", bufs=2,
                                        space="PSUM")
                    nc.tensor.matmul(p2[:pt, :], lhsT=psx[:, 0:pt],
                                     rhs=w1_t[:], start=True, stop=False)
                    nc.tensor.matmul(
                        p2[:pt, :], lhsT=xsT_t[:, t * P:t * P + pt],
                        rhs=w1_t[:], start=False, stop=True)
                    s1 = pool.tile([P, D_HID], f32, tag="s1", bufs=4)
                    nc.scalar.activation(s1[:pt, :], p2[:pt, :], AF.Copy,
                                         scale=dv)
                    s2 = pool.tile([P, D_HID], f32, tag="s2", bufs=4)
                    nc.vector.tensor_add(s2[:pt, :], s1[:pt, :], b1_t[:pt, :])
                    rp = pool.tile([P, P], bf16, tag="rp", bufs=4)
                    nc.vector.memset(rp[:, D_HID:P], 0.0)
                    nc.scalar.activation(rp[:pt, 0:D_HID], s2[:pt, :], AF.Relu,
                                         scale=dv)
                    nc.sync.dma_start(r_shp[t * P:t * P + pt, :], rp[:pt, :])
                else:
                    st = pool.tile([P, D_HID], bf16, tag="st", bufs=4)
                    nc.sync.dma_start(
                        st[:pt, :], r_shp[t * P:t * P + pt, 0:D_HID])
                    u1 = pool.tile([P, D_HID], f32, tag="u1", bufs=4)
                    nc.vector.tensor_add(u1[:pt, :], pxt[:pt, 0:D_HID], st[:pt, :])
                    u2 = pool.tile([P, D_HID], f32, tag="u2", bufs=4)
                    nc.vector.tensor_mul(u2[:pt, :], u1[:pt, :], w2_t[:pt, :])
                    yv = pool.tile([P, 1], f32, tag="yv", bufs=4)
                    nc.vector.tensor_reduce(
                        yv[:pt, :], u2[:pt, :], axis=mybir.AxisListType.X,
                        op=mybir.AluOpType.add)
                    ov = pool.tile([P, 1], f32, tag="ov", bufs=4)
                    nc.scalar.activation(ov[:pt, :], yv[:pt, :], AF.Sigmoid,
                                         bias=b2_t[:pt, :], scale=dv)
                    nc.sync.dma_start(y_out[t * P:t * P + pt, :], ov[:pt, :])

            # wait: layer-1 epilogue r' = dinv*relu(dinv*(A1@W1)+b1) --
            # the relu activation computes relu(s*x+b); we need the OUTER
            # dinv scale too, done via the rp copy scale above. See epilogue.

            gather_layer(1, xsb, n_nodes)

            nc.gpsimd.collective_compute(
                "AllGather", mybir.AluOpType.bypass, replica_groups=rg,
                ins=[r_shp[:]], outs=[r_fullp[:]])

            gather_layer(2, r_fullp, n_nodes)

    nc.compile()
    return nc


# ---------------------------------------------------------------------------


def kernel(**inputs) -> np.ndarray:
    global LAST_RESULTS
    x = np.asarray(inputs["x"], dtype=np.float32)
    edge_index = np.asarray(inputs["edge_index"])
    w1_in = np.asarray(inputs["W1"], dtype=np.float32)
    b1_in = np.asarray(inputs["b1"], dtype=np.float32)
    w2_in = np.asarray(inputs["W2"], dtype=np.float32)
    b2_in = np.asarray(inputs["b2"], dtype=np.float32)

    n_nodes = x.shape[0]
    src = edge_index[0].astype(np.int64)
    dst = edge_index[1].astype(np.int64)

    deg = np.bincount(dst, minlength=n_nodes).astype(np.float64) + 1.0
    dinv = (1.0 / np.sqrt(deg)).astype(np.float32)

    meta, per_core = _build_plan(src, dst, n_nodes, N_CORES)
    sh = meta["sh"]
    n_tiles = meta["n_tiles"]

    nc = _build_program(meta)

    xs = x * dinv[:, None]
    xsb_full = xs.astype(ml_dtypes.bfloat16)          # [N, 128]
    iota4_arr = (
        np.broadcast_to(np.arange(P, dtype=np.float32), (P, 4, P))
        .astype(ml_dtypes.bfloat16).copy())
    b1bc = np.broadcast_to(b1_in.reshape(1, D_HID), (P, D_HID)).astype(np.float32).copy()
    w2bc = np.broadcast_to(w2_in.reshape(1, D_HID), (P, D_HID)).astype(np.float32).copy()
    w1b = w1_in.astype(ml_dtypes.bfloat16)            # [128, 64]

    in_maps = []
    for c in range(N_CORES):
        idx_arr, dst_arr = per_core[c]
        xs_sh = xsb_full[c * sh:(c + 1) * sh]          # [sh, 128] bf16
        xsT = np.ascontiguousarray(xs_sh.T)            # [128, sh] bf16
        dv = np.zeros((P, n_tiles), dtype=np.float32)
        dsl = dinv[c * sh:(c + 1) * sh]
        for t in range(n_tiles):
            pt = min(P, sh - t * P)
            dv[:pt, t] = dsl[t * P:t * P + pt]
        in_maps.append({
            "xsb": xsb_full,
            "xsTb": xsT,
            "w1b": w1b,
            "b1bc": b1bc,
            "w2bc": w2bc,
            "dinv_sh": dv,
            "iota4": iota4_arr,
            "idx16": idx_arr,
            "dstloc": dst_arr,
            "b2col": np.full((P, 1), float(b2_in.reshape(-1)[0]), dtype=np.float32),
        })

    from concourse import bass_utils

    if os.environ.get("BASS_TRACE"):
        _install_axon_profile_shim()

    res = bass_utils.run_bass_kernel_spmd(
        nc, in_maps, core_ids=list(range(N_CORES)),
        trace=bool(os.environ.get("BASS_TRACE")),
        trace_cores=[0] if os.environ.get("BASS_TRACE") else None)
    LAST_RESULTS = res
    out = np.concatenate([res.results[c]["y"] for c in range(N_CORES)], axis=0)
    return out.astype(np.float32)
